# revision 19
# baseline (speedup 1.0000x reference)
"""Trainium2 Bass kernel for i1e(z) (exponentially scaled modified Bessel I1).

Input: z float32 (32, 1024, 1024), values in [0.1, 10.1] (positive).
Output: i1e(z), same shape/dtype (f32), matching the A&S-style reference to
~1.5e-2 pointwise / ~6.6e-3 norm relative error (harness gate is 2e-2).

Strategy (per core, trivially data-parallel over the leading batch axis):
  - Each of 8 cores gets 4 batches = 4Mi elements, viewed as [128, 32768] f32.
  - Single-branch approximation in the log domain:
        i1e(x) ~= exp(P4(ln x)),   P4 a quartic fit on [ln 0.1, ln 10.1]
    refit against the exact bf16-quantized evaluation chain below.  The log
    transform linearizes both asymptotics (i1e ~ x/2 near 0, ~0.4/sqrt(x)
    at inf), which is what makes a mere quartic sufficient.
  - Per tile (ScalarE ops from the natural_log_exp_and_others table set):
        u = Ln(x)                 ACT, f32 -> bf16
        t = TS(u*ALPHA + BETA)    DVE bf16 (4x mode)
        q = TT(t*t)               DVE bf16 (2x mode)
        q = (q + C)*u             DVE STT bf16 (2x, in place)
        q = (q + D)*u             DVE STT bf16 (2x, in place)
        out = Exp(q + BE)         ACT, bf16 out (f32 bias const)
    so P4 = (ALPHA*u+BETA)^2*u^2 + C*u^2 + D*u + BE spans general quartics.
  - The DRAM output is declared bf16 and upcast host-side: store bytes halve
    on both sides of the DMA, so total traffic is 16 MiB in + 8 MiB out.
    Engine rates (ACT ~(N+352)/1.2GHz dtype-independent; DVE bf16 STT/TT 2x,
    TS 4x) put ACT at ~59us and DVE at ~62us per core-pass, at/under the
    ~55-75us DMA bound - every resource rides its measured roofline.
  - Loads issue from the SP HWDGE ring, stores from the ACT HWDGE ring
    (trigger directly follows Exp on the same engine, so its wait is
    pre-satisfied and stores never head-of-line-block input loads).
"""

import numpy as np

import concourse.bass as bass
import concourse.tile as tile
from concourse import mybir
from concourse.bass_utils import run_bass_kernel_spmd

AF = mybir.ActivationFunctionType
ALU = mybir.AluOpType
F32 = mybir.dt.float32
USE_F16 = True       # IEEE fp16 for the on-chip chain + DRAM output: same
                     # 2-byte DVE perf modes as bf16 but 10 mantissa bits,
                     # cutting quantization noise ~8x (maxrel 1.54e-2 ->
                     # 9.2e-3). All chain values sit in fp16 normal range.
BF16 = mybir.dt.float16 if USE_F16 else mybir.dt.bfloat16

N_CORES = 8
P = 128              # SBUF partitions
FD_TOTAL = 32768     # free-dim elements per partition per core (4Mi total)
TILE_FD = 4096       # free-dim per tile
X_BUFS = 4           # input-tile ring depth (DMA prefetch runway)
OUT_BUFS = 3         # output-tile ring depth
TMP_BUFS = 2
STORE_SCALAR = True  # issue stores from the ACT HWDGE ring: the trigger sits
                     # right after Exp on the same engine so its wait is
                     # pre-satisfied, and stores never head-of-line-block
                     # input loads queued on the SP HWDGE ring
CAST_LOAD = False    # SWDGE (gpsimd-ring) input DMA with f32->bf16 cast.
                     # Measured NOT faster: the DMA bound tracks the f32
                     # bytes through the SDMA/HBM path regardless of the
                     # SBUF-side dtype, and the SWDGE cast path adds ~2-6us
                     # per pass, so plain HWDGE f32 loads win.
OUT_BF16 = True      # declare the DRAM output bf16 and upcast host-side:
                     # store bytes halve on BOTH sides of the DMA (unlike
                     # CAST_LOAD), cutting total DMA traffic 16+8=24 MiB and
                     # the DMA-bound floor by ~25%; Exp also reads/writes
                     # bf16 so the whole DVE chain runs in 2x/4x perf modes.
                     # Costs <=2^-9 output quantization, inside the budget.
N_ACT_SQ = 0 if OUT_BF16 else 3   # tiles (of 8) on the ACT-square path:
                     # balances ACT/DVE at ~70/70us for the f32-tail chain;
                     # the all-bf16 chain balances at ~59/62us with 0
TILE8K = False       # 8192-wide tiles: halves per-instruction overhead on
                     # ACT/DVE (~3us/pass combined) at the cost of thinner
                     # buffer rings (SBUF-limited)
IN_BF16 = False      # declare the DRAM input bf16 and cast host-side before
                     # upload (mirror of OUT_BF16): input DMA bytes halve,
                     # total traffic 8+8=16 MiB, giving the ACT-bound compute
                     # clean DMA headroom; costs ~2^-9 input quantization

# Quartic P4(u) ~= ln(i1e(e^u)) on u in [ln 0.1, ln 10.1], minimax-refit
# through the exact quantized evaluation chain for each pipeline config.
if USE_F16:
    ALPHA = 0.10348540544509888
    BETA = -0.013020294718444347
    C = -0.25046506524086
    D = 0.22452354431152344
    BE = -1.5758748054504395
elif IN_BF16:
    ALPHA = 0.1036340594291687
    BETA = -0.01279890164732933
    C = -0.25100627541542053
    D = 0.22433160245418549
    BE = -1.5729761123657227
elif OUT_BF16:
    ALPHA = 0.10368295013904572
    BETA = -0.012737303040921688
    C = -0.25116512179374695
    D = 0.22440478205680847
    BE = -1.572745680809021
elif CAST_LOAD:
    ALPHA = 0.1032966673374176
    BETA = -0.012588093057274818
    C = -0.2503528296947479
    D = 0.22434590756893158
    BE = -1.5741204023361206
else:
    ALPHA = 0.10338272154331207
    BETA = -0.012421127408742905
    C = -0.2503415644168854
    D = 0.2245168834924698
    BE = -1.5742369890213013

ACT_BIAS_CONSTS = [BETA, BE]

_CACHED_NC = None


def build_nc(reps: int = 1):
    nc = bass.Bass(trn_type="TRN2")
    x_ext = nc.declare_dram_parameter("x", [P, FD_TOTAL],
                                      BF16 if IN_BF16 else F32, isOutput=False)
    o_ext = nc.declare_dram_parameter("o", [P, FD_TOTAL],
                                      BF16 if OUT_BF16 else F32, isOutput=True)

    # Register activation-bias constants as const APs, mirroring
    # Bass.__init__'s register_const_ap for 0.0/1.0.
    for i, v in enumerate(ACT_BIAS_CONSTS):
        tns = nc.alloc_sbuf_tensor(f"const-f32-bias{i}", [P, 1], F32)
        nc.gpsimd.memset(tns.ap(), v)
        nc.const_aps.aps[(F32, v)] = tns.ap()
    nc.all_engine_barrier()

    tile_fd = 8192 if TILE8K else TILE_FD
    x_bufs, out_bufs = (2, 2) if TILE8K else (X_BUFS, OUT_BUFS)
    n_tiles = FD_TOTAL // tile_fd
    store_engine = nc.scalar if STORE_SCALAR else nc.sync
    with tile.TileContext(nc) as tc:
        with (
            tc.tile_pool(name="iox", bufs=x_bufs) as iox,
            tc.tile_pool(name="ioo", bufs=out_bufs) as ioo,
            tc.tile_pool(name="tmp", bufs=TMP_BUFS) as tmp,
        ):
            for i in range(n_tiles * reps):
                i = i % n_tiles
                sl = bass.ts(i, tile_fd)

                if CAST_LOAD:
                    x = iox.tile([P, tile_fd], BF16, tag="x")
                    nc.gpsimd.dma_start(x[:], x_ext[:, sl])
                else:
                    x = iox.tile([P, tile_fd],
                                 BF16 if IN_BF16 else F32, tag="x")
                    nc.sync.dma_start(x[:], x_ext[:, sl])

                u = tmp.tile([P, tile_fd], BF16, tag="u")
                nc.scalar.activation(u[:], x[:], AF.Ln)

                q = tmp.tile([P, tile_fd], BF16, tag="q")
                if i % 8 >= 8 - N_ACT_SQ:
                    nc.scalar.activation(q[:], u[:], AF.Square,
                                         scale=ALPHA, bias=BETA)
                else:
                    t = tmp.tile([P, tile_fd], BF16, tag="t")
                    nc.vector.tensor_scalar(t[:], u[:], ALPHA, BETA,
                                            ALU.mult, ALU.add)
                    nc.vector.tensor_tensor(q[:], t[:], t[:], ALU.mult)

                nc.vector.scalar_tensor_tensor(
                    q[:], q[:], C, u[:], ALU.add, ALU.mult)
                nc.vector.scalar_tensor_tensor(
                    q[:], q[:], D, u[:], ALU.add, ALU.mult)

                out = ioo.tile([P, tile_fd],
                               BF16 if OUT_BF16 else F32, tag="out")
                nc.scalar.activation(out[:], q[:], AF.Exp, bias=BE)

                store_engine.dma_start(o_ext[:, sl], out[:])

    _split_multi_waits(nc)
    return nc


# TPB compute-instruction ISA formats carry at most ONE sync-wait, but Tile's
# semaphore assignment can attach several (its wait minimality is per-proc,
# not transitive).  Hoist all but one wait onto an InstNoOp inserted right
# before the offending instruction on the same engine.
def _split_multi_waits(nc):
    for bb in nc.main_func.blocks:
        insts = bb.instructions
        i = 0
        while i < len(insts):
            inst = insts[i]
            si = inst.sync_info
            if si is not None and len(si.on_wait) > 1:
                for w in si.on_wait[:-1]:
                    nop = mybir.InstNoOp(
                        name=nc.get_next_instruction_name(),
                        text_hint="wait_split",
                        bass_nofuse=True,
                        engine=inst.engine,
                        sync_info=mybir.SyncInfo(on_wait=[w], on_update=[]),
                    )
                    insts.insert(i, nop)
                    i += 1
                si.on_wait = [si.on_wait[-1]]
            i += 1


def kernel(z: np.ndarray) -> np.ndarray:
    global _CACHED_NC
    assert z.shape == (32, 1024, 1024) and z.dtype == np.float32
    if _CACHED_NC is None:
        _CACHED_NC = build_nc()
    nc = _CACHED_NC

    per_core = 32 // N_CORES
    shards = z.reshape(N_CORES, per_core * 1024 * 1024).reshape(N_CORES, P, FD_TOTAL)
    if IN_BF16:
        from ml_dtypes import bfloat16
        shards = shards.astype(bfloat16)
    in_maps = [{"x": np.ascontiguousarray(shards[k])} for k in range(N_CORES)]
    res = run_bass_kernel_spmd(nc, in_maps, list(range(N_CORES))).results
    out = np.concatenate(
        [res[k]["o"].astype(np.float32).reshape(per_core, 1024, 1024)
         for k in range(N_CORES)], axis=0
    )
    return out.astype(np.float32)


# revision 20
# speedup vs baseline: 1.0354x; 1.0354x over previous
"""Trainium2 Bass kernel for i1e(z) (exponentially scaled modified Bessel I1).

Input: z float32 (32, 1024, 1024), values in [0.1, 10.1] (positive).
Output: i1e(z), same shape/dtype (f32), matching the A&S-style reference to
~9.5e-3 pointwise / ~5.9e-3 norm relative error (harness gate is 2e-2).
The 16-bit on-chip dtype is IEEE fp16 (USE_F16), not bf16: same 2-byte DVE
perf modes, 8x less quantization noise on this narrow-range data.

Strategy (per core, trivially data-parallel over the leading batch axis):
  - Each of 8 cores gets 4 batches = 4Mi elements, viewed as [128, 32768] f32.
  - Single-branch approximation in the log domain:
        i1e(x) ~= exp(P4(ln x)),   P4 a quartic fit on [ln 0.1, ln 10.1]
    refit against the exact bf16-quantized evaluation chain below.  The log
    transform linearizes both asymptotics (i1e ~ x/2 near 0, ~0.4/sqrt(x)
    at inf), which is what makes a mere quartic sufficient.
  - Per tile (ScalarE ops from the natural_log_exp_and_others table set):
        u = Ln(x)                 ACT, f32 -> bf16
        t = TS(u*ALPHA + BETA)    DVE bf16 (4x mode)
        q = TT(t*t)               DVE bf16 (2x mode)
        q = (q + C)*u             DVE STT bf16 (2x, in place)
        q = (q + D)*u             DVE STT bf16 (2x, in place)
        out = Exp(q + BE)         ACT, bf16 out (f32 bias const)
    so P4 = (ALPHA*u+BETA)^2*u^2 + C*u^2 + D*u + BE spans general quartics.
  - The DRAM output is declared bf16 and upcast host-side: store bytes halve
    on both sides of the DMA, so total traffic is 16 MiB in + 8 MiB out.
    Engine rates (ACT ~(N+352)/1.2GHz dtype-independent; DVE bf16 STT/TT 2x,
    TS 4x) put ACT at ~59us and DVE at ~62us per core-pass, at/under the
    ~55-75us DMA bound - every resource rides its measured roofline.
  - Loads issue from the SP HWDGE ring, stores from the ACT HWDGE ring
    (trigger directly follows Exp on the same engine, so its wait is
    pre-satisfied and stores never head-of-line-block input loads).
"""

import numpy as np

import concourse.bass as bass
import concourse.tile as tile
from concourse import mybir
from concourse.bass_utils import run_bass_kernel_spmd

AF = mybir.ActivationFunctionType
ALU = mybir.AluOpType
F32 = mybir.dt.float32
USE_F16 = True       # IEEE fp16 for the on-chip chain + DRAM output: same
                     # 2-byte DVE perf modes as bf16 but 10 mantissa bits,
                     # cutting quantization noise ~8x (maxrel 1.54e-2 ->
                     # 9.2e-3). All chain values sit in fp16 normal range.
BF16 = mybir.dt.float16 if USE_F16 else mybir.dt.bfloat16

N_CORES = 8
P = 128              # SBUF partitions
FD_TOTAL = 32768     # free-dim elements per partition per core (4Mi total)
TILE_FD = 4096       # free-dim per tile
X_BUFS = 4           # input-tile ring depth (DMA prefetch runway)
OUT_BUFS = 3         # output-tile ring depth
TMP_BUFS = 2
STORE_SCALAR = True  # issue stores from the ACT HWDGE ring: the trigger sits
                     # right after Exp on the same engine so its wait is
                     # pre-satisfied, and stores never head-of-line-block
                     # input loads queued on the SP HWDGE ring
CAST_LOAD = False    # SWDGE (gpsimd-ring) input DMA with f32->bf16 cast.
                     # Measured NOT faster: the DMA bound tracks the f32
                     # bytes through the SDMA/HBM path regardless of the
                     # SBUF-side dtype, and the SWDGE cast path adds ~2-6us
                     # per pass, so plain HWDGE f32 loads win.
OUT_BF16 = True      # declare the DRAM output bf16 and upcast host-side:
                     # store bytes halve on BOTH sides of the DMA (unlike
                     # CAST_LOAD), cutting total DMA traffic 16+8=24 MiB and
                     # the DMA-bound floor by ~25%; Exp also reads/writes
                     # bf16 so the whole DVE chain runs in 2x/4x perf modes.
                     # Costs <=2^-9 output quantization, inside the budget.
N_ACT_SQ = 0 if OUT_BF16 else 3   # tiles (of 8) on the ACT-square path:
                     # balances ACT/DVE at ~70/70us for the f32-tail chain;
                     # the all-bf16 chain balances at ~59/62us with 0
TILE8K = False       # 8192-wide tiles: halves per-instruction overhead on
                     # ACT/DVE (~3us/pass combined) at the cost of thinner
                     # buffer rings (SBUF-limited)
IN_BF16 = False      # declare the DRAM input bf16 and cast host-side before
                     # upload (mirror of OUT_BF16): input DMA bytes halve,
                     # total traffic 8+8=16 MiB, giving the ACT-bound compute
                     # clean DMA headroom; costs ~2^-9 input quantization

# Quartic P4(u) ~= ln(i1e(e^u)) on u in [ln 0.1, ln 10.1], minimax-refit
# through the exact quantized evaluation chain for each pipeline config.
if USE_F16:
    ALPHA = 0.10348540544509888
    BETA = -0.013020294718444347
    C = -0.25046506524086
    D = 0.22452354431152344
    BE = -1.5758748054504395
elif IN_BF16:
    ALPHA = 0.1036340594291687
    BETA = -0.01279890164732933
    C = -0.25100627541542053
    D = 0.22433160245418549
    BE = -1.5729761123657227
elif OUT_BF16:
    ALPHA = 0.10368295013904572
    BETA = -0.012737303040921688
    C = -0.25116512179374695
    D = 0.22440478205680847
    BE = -1.572745680809021
elif CAST_LOAD:
    ALPHA = 0.1032966673374176
    BETA = -0.012588093057274818
    C = -0.2503528296947479
    D = 0.22434590756893158
    BE = -1.5741204023361206
else:
    ALPHA = 0.10338272154331207
    BETA = -0.012421127408742905
    C = -0.2503415644168854
    D = 0.2245168834924698
    BE = -1.5742369890213013

ACT_BIAS_CONSTS = [BETA, BE]

_CACHED_NC = None


def build_nc(reps: int = 1):
    nc = bass.Bass(trn_type="TRN2")
    x_ext = nc.declare_dram_parameter("x", [P, FD_TOTAL],
                                      BF16 if IN_BF16 else F32, isOutput=False)
    o_ext = nc.declare_dram_parameter("o", [P, FD_TOTAL],
                                      BF16 if OUT_BF16 else F32, isOutput=True)

    # Register activation-bias constants as const APs, mirroring
    # Bass.__init__'s register_const_ap for 0.0/1.0.
    for i, v in enumerate(ACT_BIAS_CONSTS):
        tns = nc.alloc_sbuf_tensor(f"const-f32-bias{i}", [P, 1], F32)
        nc.gpsimd.memset(tns.ap(), v)
        nc.const_aps.aps[(F32, v)] = tns.ap()
    nc.all_engine_barrier()

    tile_fd = 8192 if TILE8K else TILE_FD
    x_bufs, out_bufs = (2, 2) if TILE8K else (X_BUFS, OUT_BUFS)
    n_tiles = FD_TOTAL // tile_fd
    store_engine = nc.scalar if STORE_SCALAR else nc.sync
    with tile.TileContext(nc) as tc:
        with (
            tc.tile_pool(name="iox", bufs=x_bufs) as iox,
            tc.tile_pool(name="ioo", bufs=out_bufs) as ioo,
            tc.tile_pool(name="tmp", bufs=TMP_BUFS) as tmp,
        ):
            for i in range(n_tiles * reps):
                i = i % n_tiles
                sl = bass.ts(i, tile_fd)

                if CAST_LOAD:
                    x = iox.tile([P, tile_fd], BF16, tag="x")
                    nc.gpsimd.dma_start(x[:], x_ext[:, sl])
                else:
                    x = iox.tile([P, tile_fd],
                                 BF16 if IN_BF16 else F32, tag="x")
                    nc.sync.dma_start(x[:], x_ext[:, sl])

                u = tmp.tile([P, tile_fd], BF16, tag="u")
                nc.scalar.activation(u[:], x[:], AF.Ln)

                q = tmp.tile([P, tile_fd], BF16, tag="q")
                if i % 8 >= 8 - N_ACT_SQ:
                    nc.scalar.activation(q[:], u[:], AF.Square,
                                         scale=ALPHA, bias=BETA)
                else:
                    t = tmp.tile([P, tile_fd], BF16, tag="t")
                    nc.vector.tensor_scalar(t[:], u[:], ALPHA, BETA,
                                            ALU.mult, ALU.add)
                    nc.vector.tensor_tensor(q[:], t[:], t[:], ALU.mult)

                nc.vector.scalar_tensor_tensor(
                    q[:], q[:], C, u[:], ALU.add, ALU.mult)
                nc.vector.scalar_tensor_tensor(
                    q[:], q[:], D, u[:], ALU.add, ALU.mult)

                out = ioo.tile([P, tile_fd],
                               BF16 if OUT_BF16 else F32, tag="out")
                nc.scalar.activation(out[:], q[:], AF.Exp, bias=BE)

                store_engine.dma_start(o_ext[:, sl], out[:])

    _split_multi_waits(nc)
    return nc


# TPB compute-instruction ISA formats carry at most ONE sync-wait, but Tile's
# semaphore assignment can attach several (its wait minimality is per-proc,
# not transitive).  Hoist all but one wait onto an InstNoOp inserted right
# before the offending instruction on the same engine.
def _split_multi_waits(nc):
    for bb in nc.main_func.blocks:
        insts = bb.instructions
        i = 0
        while i < len(insts):
            inst = insts[i]
            si = inst.sync_info
            if si is not None and len(si.on_wait) > 1:
                for w in si.on_wait[:-1]:
                    nop = mybir.InstNoOp(
                        name=nc.get_next_instruction_name(),
                        text_hint="wait_split",
                        bass_nofuse=True,
                        engine=inst.engine,
                        sync_info=mybir.SyncInfo(on_wait=[w], on_update=[]),
                    )
                    insts.insert(i, nop)
                    i += 1
                si.on_wait = [si.on_wait[-1]]
            i += 1


def kernel(z: np.ndarray) -> np.ndarray:
    global _CACHED_NC
    assert z.shape == (32, 1024, 1024) and z.dtype == np.float32
    if _CACHED_NC is None:
        _CACHED_NC = build_nc()
    nc = _CACHED_NC

    per_core = 32 // N_CORES
    shards = z.reshape(N_CORES, per_core * 1024 * 1024).reshape(N_CORES, P, FD_TOTAL)
    if IN_BF16:
        from ml_dtypes import bfloat16
        shards = shards.astype(bfloat16)
    in_maps = [{"x": np.ascontiguousarray(shards[k])} for k in range(N_CORES)]
    res = run_bass_kernel_spmd(nc, in_maps, list(range(N_CORES))).results
    out = np.concatenate(
        [res[k]["o"].astype(np.float32).reshape(per_core, 1024, 1024)
         for k in range(N_CORES)], axis=0
    )
    return out.astype(np.float32)
